# revision 8
# baseline (speedup 1.0000x reference)
"""Multi-head causal attention on 8 Trainium2 NeuronCores.

Sharding: tensor-parallel over heads x data-parallel over batch.
Core c handles batch c//4 and heads [4*(c%4), 4*(c%4)+4). Each core
computes Q/K/V projections for its head slice over the full sequence,
causal attention (transposed scores, ones-column softmax denominator),
and a partial output projection against its row-slice of W_o. The 4
partial outputs per batch are summed on the host (the all-reduce of
row-parallel W_o), which also adds b_o.

Schedule: software-pipelined per 512-wide query block. The xt input is
DMA'd in 4 per-block chunks so the first projection starts ~5us in.
Attention(qb-1) is woven into the PE/ACT/DVE instruction streams with
projections(qb) so the scalar engine's exp work overlaps the tensor
engine's projection matmuls. Output projections for qb 0..2 are
deferred to the tail where they overlap attention(qb=3).
"""
import sys

sys.path.insert(0, '/opt/trn_rl_repo')

import numpy as np
import ml_dtypes

B, S, D, H, DK = 2, 2048, 1024, 16, 64
NCORES = 8
HL = 4            # heads per core
DL = HL * DK      # head-dim slice per core (256)
NQB = S // 512    # 512-wide query blocks
NKST = S // 128   # 128-wide key tiles

_cache = {}


def _weave(streams):
    """Merge several ordered lists of thunks proportionally and run them."""
    items = []
    for si, s in enumerate(streams):
        L = len(s)
        for j, fn in enumerate(s):
            items.append(((j + 0.5) / L, si, fn))
    items.sort(key=lambda t: (t[0], t[1]))
    for _, _, fn in items:
        fn()


def _build(repeat=1, dynamic=False, stage=4):
    """stage: 1=DMAs only, 2=+projections, 3=+attention, 4=full."""
    import concourse.bacc as bacc
    import concourse.mybir as mybir
    import concourse.tile as tile
    from contextlib import ExitStack, nullcontext

    f32, bf16 = mybir.dt.float32, mybir.dt.bfloat16
    Ident, Exp = mybir.ActivationFunctionType.Identity, mybir.ActivationFunctionType.Exp

    nc = bacc.Bacc("TRN2", target_bir_lowering=False, debug=False, num_devices=NCORES)
    xt_d = nc.dram_tensor("xt", (D, S), bf16, kind="ExternalInput").ap()
    wq_d = nc.dram_tensor("wq", (D, DL), bf16, kind="ExternalInput").ap()
    wk_d = nc.dram_tensor("wk", (D, DL), bf16, kind="ExternalInput").ap()
    wv_d = nc.dram_tensor("wv", (D, DL), bf16, kind="ExternalInput").ap()
    wo_d = nc.dram_tensor("wo", (DL, D), bf16, kind="ExternalInput").ap()
    bqk_d = nc.dram_tensor("bqk", (DL, 2), f32, kind="ExternalInput").ap()
    bv_d = nc.dram_tensor("bv", (DL,), f32, kind="ExternalInput").ap()
    mask_d = nc.dram_tensor("masks", (4, 128, 512), bf16, kind="ExternalInput").ap()
    po_d = nc.dram_tensor("po", (128, NQB, 8, 512), bf16, kind="ExternalOutput").ap()

    with tile.TileContext(nc) as tc:
        with ExitStack() as ctx:
            sb = ctx.enter_context(tc.tile_pool(name="sb", bufs=1))
            ps = ctx.enter_context(tc.tile_pool(name="ps", bufs=1, space="PSUM"))

            # ---- persistent SBUF tiles ----
            xt = sb.tile([128, 8, S], bf16, name="xt")
            wq_s = sb.tile([128, 8, DL], bf16, name="wq_s")
            wk_s = sb.tile([128, 8, DL], bf16, name="wk_s")
            wv_s = sb.tile([128, 8, DL], bf16, name="wv_s")
            wo_s = sb.tile([128, 2, D], bf16, name="wo_s")
            qt = [sb.tile([128, S], bf16, name=f"qt{p}") for p in range(2)]
            kt = [sb.tile([128, S], bf16, name=f"kt{p}") for p in range(2)]
            ctxt = [sb.tile([128, S], bf16, name=f"ctxt{p}") for p in range(2)]
            # v_aug: [128, ks-tile, 2 pairs x (64 even | one | spare | 64 odd | one | spare)]
            v_aug = sb.tile([128, NKST, 264], bf16, name="v_aug")
            masks = sb.tile([128, 4, 512], bf16, name="masks")
            bqk_t = sb.tile([128, 2, 2], f32, name="bqk_t")
            bv_sb = sb.tile([1, DL], f32, name="bv_sb")
            bvB = sb.tile([128, DL], f32, name="bvB")

            rep_ctx = tc.For_i(0, repeat, 1) if dynamic else nullcontext(range(repeat))
            with rep_ctx as _it:
              for _rep in ([0] if dynamic else _it):
                # ---- input DMAs: sync ring: xt chunks + po out; scalar: weights ----
                xt_r = xt_d.rearrange("(k p) s -> p k s", p=128)
                nc.sync.dma_start(xt[:, 0:4, 0:512], xt_r[:, 0:4, 0:512])
                nc.sync.dma_start(xt[:, 4:8, 0:512], xt_r[:, 4:8, 0:512])
                for qb in range(1, NQB):
                    sl = slice(qb * 512, (qb + 1) * 512)
                    nc.sync.dma_start(xt[:, :, sl], xt_r[:, :, sl])
                nc.scalar.dma_start(wk_s[:], wk_d.rearrange("(k p) n -> p k n", p=128))
                nc.scalar.dma_start(wv_s[:], wv_d.rearrange("(k p) n -> p k n", p=128))
                nc.scalar.dma_start(wq_s[:], wq_d.rearrange("(k p) n -> p k n", p=128))
                nc.scalar.dma_start(wo_s[:], wo_d.rearrange("(k p) n -> p k n", p=128))
                nc.gpsimd.dma_start(bqk_t[:], bqk_d.rearrange("(p2 p) j -> p p2 j", p=128))
                nc.gpsimd.dma_start(bv_sb[:], bv_d.rearrange("(o n) -> o n", o=1))
                nc.gpsimd.dma_start(masks[:], mask_d.rearrange("t p n -> p t n"))
                nc.gpsimd.partition_broadcast(bvB[:], bv_sb[:])

                # ---------- chunk emitters ----------
                def qk_chunk(dst, w_s, b_t, p, qb):
                    def run():
                        sl = slice(qb * 512, (qb + 1) * 512)
                        pp = ps.tile([128, 512], f32, tag="proj", bufs=2)
                        for k in range(8):
                            nc.tensor.matmul(pp[:], w_s[:, k, p * 128:(p + 1) * 128],
                                             xt[:, k, sl],
                                             start=(k == 0), stop=(k == 7))
                        nc.vector.tensor_scalar_add(dst[p][:, sl], pp[:],
                                                    bqk_t[:, p, b_t:b_t + 1])
                    return run

                def v_chunk(sp):
                    def run():
                        pv = ps.tile([128, 512], f32, tag="proj", bufs=2)
                        for k in range(8):
                            nc.tensor.matmul(pv[:, 0:DL], xt[:, k, sp * 128:(sp + 1) * 128],
                                             wv_s[:, k, :], start=(k == 0), stop=(k == 7))
                        vdst = v_aug[:, sp, :].rearrange("p (pr e q) -> p pr e q", pr=2, e=2, q=66)
                        nc.vector.tensor_add(vdst[:, :, :, 0:64],
                                             pv[:, 0:DL].rearrange("p (pr e q) -> p pr e q", pr=2, e=2, q=64),
                                             bvB[:].rearrange("p (pr e q) -> p pr e q", pr=2, e=2, q=64))
                        nc.gpsimd.memset(vdst[:, :, :, 64:65], 1.0)
                    return run

                def proj_chunks(qb):
                    out = []
                    if stage < 2:
                        return out
                    for p in range(2):
                        out.append(qk_chunk(kt, wk_s, 1, p, qb))
                    for sp in range(4 * qb, 4 * qb + 4):
                        out.append(v_chunk(sp))
                    for p in range(2):
                        out.append(qk_chunk(qt, wq_s, 0, p, qb))
                    return out

                # attention state: per (p, qb) the av PSUM tile + e tiles
                def att_chunks(qb):
                    """List of thunks: per p: sc(g) steps pipelined one gen
                    ahead of av(g) steps, then a normalize step."""
                    if stage < 3:
                        return []
                    n_kst = 4 * qb + 4
                    n_grp = n_kst // 2
                    steps = []
                    for p in range(2):
                        st = {"av": None, "e": {}}

                        def sc_step(p=p, g=0, qb=qb, st=st):
                            def run():
                                sc0 = ps.tile([128, 1024], f32, tag="sc", bufs=2)
                                sc1 = ps.tile([128, 1024], f32, tag="sc", bufs=2)
                                for i in range(2):
                                    kst = 2 * g + i
                                    nc.tensor.matmul(sc0[:, i * 512:(i + 1) * 512],
                                                     kt[p][0:64, kst * 128:(kst + 1) * 128],
                                                     qt[p][0:64, qb * 512:(qb + 1) * 512],
                                                     start=True, stop=True, tile_position=(0, 0))
                                    nc.tensor.matmul(sc1[:, i * 512:(i + 1) * 512],
                                                     kt[p][64:128, kst * 128:(kst + 1) * 128],
                                                     qt[p][64:128, qb * 512:(qb + 1) * 512],
                                                     start=True, stop=True, tile_position=(64, 0))
                                e0 = sb.tile([128, 1024], bf16, tag="ex", bufs=4)
                                e1 = sb.tile([128, 1024], bf16, tag="ex", bufs=4)
                                nc.scalar.activation(e0[:], sc0[:], Exp, scale=0.125)
                                nc.scalar.activation(e1[:], sc1[:], Exp, scale=0.125)
                                if g >= 2 * qb:         # diagonal group: causal mask
                                    h = g - 2 * qb
                                    m = masks[:, 2 * h:2 * h + 2, :].rearrange("p t n -> p (t n)")
                                    nc.vector.tensor_mul(e0[:], e0[:], m)
                                    nc.vector.tensor_mul(e1[:], e1[:], m)
                                st["e"][g] = (e0, e1)
                            return run

                        def av_step(p=p, g=0, qb=qb, st=st, n_grp=n_grp):
                            def run():
                                if st["av"] is None:
                                    st["av"] = ps.tile([65, 1024], f32, tag="av", bufs=1, name="av")
                                av = st["av"]
                                e0, e1 = st["e"].pop(g)
                                for i in range(2):
                                    kst = 2 * g + i
                                    first, last = (g == 0 and i == 0), (g == n_grp - 1 and i == 1)
                                    nc.tensor.matmul(av[:, 0:512], v_aug[:, kst, p * 132:p * 132 + 65],
                                                     e0[:, i * 512:(i + 1) * 512], start=first, stop=last)
                                    nc.tensor.matmul(av[:, 512:1024], v_aug[:, kst, p * 132 + 66:p * 132 + 131],
                                                     e1[:, i * 512:(i + 1) * 512], start=first, stop=last)
                            return run

                        def norm_step(p=p, qb=qb, st=st):
                            def run():
                                av = st["av"]
                                rc = sb.tile([1, 1024], f32, tag="rc", bufs=2)
                                rb = sb.tile([64, 1024], f32, tag="rb", bufs=2)
                                nc.vector.reciprocal(rc[:], av[64:65, :])
                                nc.gpsimd.partition_broadcast(rb[:], rc[:])
                                for e in range(2):
                                    nc.vector.tensor_mul(
                                        ctxt[p][e * 64:(e + 1) * 64, qb * 512:(qb + 1) * 512],
                                        av[0:64, e * 512:(e + 1) * 512],
                                        rb[:, e * 512:(e + 1) * 512])
                            return run

                        prev = None
                        for g in range(n_grp):
                            steps.append(sc_step(p=p, g=g))
                            if prev is not None:
                                steps.append(av_step(p=p, g=prev))
                            prev = g
                        steps.append(av_step(p=p, g=prev))
                        steps.append(norm_step(p=p))
                    return steps

                def outproj_chunks(qb):
                    if stage < 4:
                        return []
                    po_sb = sb.tile([128, 8, 512], bf16, tag="po_s", bufs=2)
                    out = []

                    def ot_chunk(ot, qb=qb, po_sb=po_sb):
                        def run():
                            po_p = ps.tile([128, 512], f32, tag="proj", bufs=2)
                            for k in range(2):
                                nc.tensor.matmul(po_p[:], wo_s[:, k, ot * 128:(ot + 1) * 128],
                                                 ctxt[k][:, qb * 512:(qb + 1) * 512],
                                                 start=(k == 0), stop=(k == 1))
                            if qb == NQB - 1 and ot % 2 == 0:
                                nc.scalar.activation(po_sb[:, ot, :], po_p[:], Ident)
                            else:
                                nc.vector.tensor_copy(po_sb[:, ot, :], po_p[:])
                        return run

                    def dma_out(half, qb=qb, po_sb=po_sb):
                        def run():
                            sl = slice(half * 4, half * 4 + 4)
                            nc.sync.dma_start(po_d[:, qb, sl, :], po_sb[:, sl, :])
                        return run

                    for ot in range(4):
                        out.append(ot_chunk(ot))
                    out.append(dma_out(0))
                    for ot in range(4, 8):
                        out.append(ot_chunk(ot))
                    out.append(dma_out(1))
                    return out

                # ---------- emission schedule ----------
                for fn in proj_chunks(0):
                    fn()
                for qb in range(1, NQB):
                    _weave([att_chunks(qb - 1), proj_chunks(qb)])
                deferred = []
                for qb in range(NQB - 1):
                    deferred.extend(outproj_chunks(qb))
                # hold back a few chunks to cover the final normalize latency
                hold = deferred[-5:] if stage >= 4 else []
                _weave([att_chunks(NQB - 1), deferred[:-5] if stage >= 4 else deferred])
                for fn in hold:
                    fn()
                for fn in outproj_chunks(NQB - 1):
                    fn()

    nc.compile()
    return nc


def _causal_mask_ok(mask):
    m = np.asarray(mask)
    if m.shape != (S, S):
        return False
    return np.array_equal(m.astype(bool), np.triu(np.ones((S, S), bool), k=1))


def _numpy_fallback(x, mask, Wq, bq, Wk, bk, Wv, bv, Wo, bo):
    x = np.asarray(x, np.float64)
    q = (x @ Wq + bq).reshape(B, S, H, DK).transpose(0, 2, 1, 3)
    k = (x @ Wk + bk).reshape(B, S, H, DK).transpose(0, 2, 1, 3)
    v = (x @ Wv + bv).reshape(B, S, H, DK).transpose(0, 2, 1, 3)
    s = np.einsum("bhqd,bhkd->bhqk", q, k) / np.sqrt(DK)
    s = np.where(np.asarray(mask, bool), -np.inf, s)
    s = s - s.max(-1, keepdims=True)
    e = np.exp(s)
    a = e / e.sum(-1, keepdims=True)
    ctx = np.einsum("bhqk,bhkd->bhqd", a, v).transpose(0, 2, 1, 3).reshape(B, S, D)
    return (ctx @ Wo + bo).astype(np.float32)


def _tri_masks():
    m = np.zeros((4, 128, 512), np.float32)
    n = np.arange(512)
    for t in range(4):
        for p_ in range(128):
            m[t, p_, :] = (n >= t * 128 + p_)
    return m.astype(ml_dtypes.bfloat16)


def _make_in_maps(x, Wq, bq, Wk, bk, Wv, bv, Wo):
    Wq, Wk, Wv, Wo = (np.asarray(w, np.float32) for w in (Wq, Wk, Wv, Wo))
    bq, bk, bv = (np.asarray(b_, np.float32) for b_ in (bq, bk, bv))
    masks_np = _tri_masks()
    xts = [np.ascontiguousarray(x[b_].T.astype(ml_dtypes.bfloat16)) for b_ in range(B)]

    in_maps = []
    for c in range(NCORES):
        b_, hs = c // 4, (c % 4) * DL
        in_maps.append({
            "xt": xts[b_],
            "wq": np.ascontiguousarray(Wq[:, hs:hs + DL].astype(ml_dtypes.bfloat16)),
            "wk": np.ascontiguousarray(Wk[:, hs:hs + DL].astype(ml_dtypes.bfloat16)),
            "wv": np.ascontiguousarray(Wv[:, hs:hs + DL].astype(ml_dtypes.bfloat16)),
            "wo": np.ascontiguousarray(Wo[hs:hs + DL, :].astype(ml_dtypes.bfloat16)),
            "bqk": np.ascontiguousarray(np.stack([bq[hs:hs + DL], bk[hs:hs + DL]], 1)),
            "bv": np.ascontiguousarray(bv[hs:hs + DL]),
            "masks": masks_np,
        })
    return in_maps


def kernel(x, mask, Wq, bq, Wk, bk, Wv, bv, Wo, bo):
    x = np.ascontiguousarray(np.asarray(x, np.float32))
    if not _causal_mask_ok(mask):
        return _numpy_fallback(x, mask, Wq, bq, Wk, bk, Wv, bv, Wo, bo)

    from concourse import bass_utils

    if "nc" not in _cache:
        _cache["nc"] = _build(repeat=1)
    nc = _cache["nc"]

    bo = np.asarray(bo, np.float32)
    in_maps = _make_in_maps(x, Wq, bq, Wk, bk, Wv, bv, Wo)

    res = bass_utils.run_bass_kernel_spmd(nc, in_maps, core_ids=list(range(NCORES)))

    out = np.empty((B, S, D), np.float32)
    for b_ in range(B):
        acc = res.results[b_ * 4]["po"].astype(np.float32)
        for g in range(1, 4):
            acc = acc + res.results[b_ * 4 + g]["po"]
        # acc[p, qb, k, s] = outT[k*128+p, qb*512+s]
        out[b_] = acc.transpose(1, 3, 2, 0).reshape(S, D) + bo
    return out
